# revision 44
# baseline (speedup 1.0000x reference)
"""Trainium2 Bass kernel for nn_Block_mamba (Mamba block + EinFFT block).

Sharding over 8 NeuronCores: core c -> batch b = c // 4, state-block j = c % 4.
Each core runs the full front-end for its batch (LN1, in_proj, causal depthwise
conv, x_proj, dt_proj) in fp16/bf16 matmul precision, scans its 16 SSM states
over all 512 channels with the DVE tensor_tensor_scan instruction (fp16
operands), forms the partial out-projection, AllReduces it within the 4-core
batch group in 4 token chunks (overlapped with compute), then computes LN2 +
its EinFFT output block entirely in SBUF with bf16 matmuls (fft2 mixes the
token axis and the 4-block axis; the length-4 block DFT is folded into
constant matrices; the 2048-point token DFT is factorized as 16 x 128 with
twiddles folded into per-r constant matrices).

Self-contained: hardcodes all shapes; host-side code is numpy only.
"""
import os
import sys
import numpy as np

for _p in ("/opt/trn_rl_repo", os.path.expanduser("~/.axon_site/_ro/trn_rl_repo")):
    if os.path.isdir(_p) and _p not in sys.path:
        sys.path.insert(0, _p)

import ml_dtypes
import concourse.bass as bass
import concourse.tile as tile
from concourse import mybir
from concourse.bass_utils import run_bass_kernel_spmd

FP = mybir.dt.float32
FP16 = mybir.dt.float16
BF16 = mybir.dt.bfloat16
ALU = mybir.AluOpType
AF = mybir.ActivationFunctionType

DIM = 256
D_STATE = 64
D_CONV = 4
D_INNER = 512
DT_RANK = 16
NB = 4
BS = 64
B_SZ = 2
L = 2048
LAM = 0.01
NDT = D_INNER // 128          # 4 d-tiles
NTT = L // 128                # 16 t-tiles
SLOC = 16                     # states per core
EPS = 1e-5
NAR = 4                       # AllReduce chunks (tokens)

F16 = np.float16
B16 = ml_dtypes.bfloat16


# --------------------------------------------------------------------------
# host-side constants
# --------------------------------------------------------------------------

def _shared_consts(W):
    c = {}
    c["w_in_t"] = np.ascontiguousarray(
        W["in_proj_w"].T.reshape(2, 128, 2 * D_INNER)).astype(F16)
    c["w_dtraw_t"] = np.ascontiguousarray(
        W["x_proj_w"][:DT_RANK].T.reshape(NDT, 128, DT_RANK)).astype(F16)
    c["w_dt_t"] = np.ascontiguousarray(W["dt_proj_w"].T).astype(F16)  # (16,512)
    c["w_out_t"] = np.ascontiguousarray(
        W["out_proj_w"].T.reshape(NDT, 128, DIM)).astype(F16)
    cwr = W["conv_w"][:, 0, :].reshape(NDT, 128, D_CONV)
    cdiag = np.zeros((NDT, D_CONV, 128, 128), np.float32)
    for i in range(NDT):
        for k in range(D_CONV):
            np.fill_diagonal(cdiag[i, k], cwr[i, :, k])
    c["conv_diag"] = cdiag.astype(F16)
    c["conv_b"] = np.ascontiguousarray(
        W["conv_b"].reshape(NDT, 128, 1)).astype(np.float32)
    c["dt_b"] = np.ascontiguousarray(
        W["dt_proj_b"].reshape(NDT, 128, 1)).astype(np.float32)
    c["d_q"] = np.ascontiguousarray(
        (W["D"] / 4.0).reshape(NDT, 128, 1)).astype(np.float32)  # /4: AllReduce sums 4x
    c["norm1_w_bc"] = np.ascontiguousarray(
        np.broadcast_to(W["norm1_w"][None, :], (128, DIM))).astype(F16)
    c["norm1_b_bc"] = np.ascontiguousarray(
        np.broadcast_to(W["norm1_b"][None, :], (128, DIM))).astype(F16)
    c["norm2_w_bc"] = np.ascontiguousarray(
        np.broadcast_to(W["norm2_w"][None, :], (128, DIM))).astype(B16)
    c["norm2_b_bc"] = np.ascontiguousarray(
        np.broadcast_to(W["norm2_b"][None, :], (128, DIM))).astype(B16)
    c["ident128"] = np.eye(128, dtype=F16)
    sel = np.zeros((SLOC, SLOC, 128), np.float32)
    for si in range(SLOC):
        sel[si, si, :] = 1.0
    c["sel16"] = np.ascontiguousarray(
        sel.transpose(1, 0, 2).reshape(SLOC, SLOC * 128)).astype(F16)
    # sel16[k, si*128+m] = 1 iff k == si  -> lhsT slice (16,128) per si

    # ---- EinFFT constants (all bf16 matmul operands)
    a_i = np.arange(16)
    # G-stage lhsT[(4a+k), (16kf+r)] = e^{-2pi i a r/16} * 1/2 * e^{-2pi i kf k/4}
    Mc = np.zeros((64, 64), np.complex128)
    for a in range(16):
        for k in range(NB):
            for kf in range(NB):
                for r in range(16):
                    Mc[4 * a + k, 16 * kf + r] = 0.5 * np.exp(
                        -2j * np.pi * (a * r / 16.0 + kf * k / 4.0))
    c["g_big"] = np.ascontiguousarray(
        np.concatenate([Mc.real, Mc.imag], axis=1)).astype(B16)   # (64,128)

    p = np.arange(128)
    s = np.arange(128)
    are = np.zeros((16, 128, 128), np.float32)
    aim = np.zeros((16, 128, 128), np.float32)
    iare = np.zeros((16, 128, 128), np.float32)
    iaim = np.zeros((16, 128, 128), np.float32)
    for r in range(16):
        ph = np.outer(s, p) / 128.0 + p[None, :] * r / 2048.0      # [s,p]
        Ar = np.exp(-2j * np.pi * ph) / np.sqrt(L)                  # A_r[s,p]
        are[r] = Ar.real.T.astype(np.float32)                       # A_r^T [p,s]
        aim[r] = Ar.imag.T.astype(np.float32)
        iph = np.outer(p, s) / 128.0 + p[:, None] * r / 2048.0      # [p,s]
        IAr = np.exp(2j * np.pi * iph) / np.sqrt(L)                 # IA_r[p,s]
        iare[r] = IAr.real.T.astype(np.float32)                     # IA_r^T [s,p]
        iaim[r] = IAr.imag.T.astype(np.float32)
    c["fft_are"] = are.astype(B16)
    c["fft_aim"] = aim.astype(B16)
    c["fft_naim"] = (-aim).astype(B16)
    c["fft_iare"] = iare.astype(B16)
    c["fft_iaim"] = iaim.astype(B16)
    c["fft_niaim"] = (-iaim).astype(B16)

    c1r = W["cw1"][0].astype(np.float64)   # (4,64,64) [d,o]
    c1i = W["cw1"][1].astype(np.float64)
    c2r = W["cw2"][0].astype(np.float64)
    c2i = W["cw2"][1].astype(np.float64)
    # layer-1 lhsT, duplicated into both partition halves so the lhsT base
    # partition can match the rhs (ft tiles hold kf pairs stacked on
    # partitions; bass requires equal base partitions).
    l1a = np.concatenate([c1r, c1i], axis=2)                       # (4,64,128)
    l1b = np.concatenate([-c1i, c1r], axis=2)
    c["mlp1a"] = np.ascontiguousarray(
        np.concatenate([l1a, l1a], axis=1)).astype(B16)            # (4,128,128)
    c["mlp1b"] = np.ascontiguousarray(
        np.concatenate([l1b, l1b], axis=1)).astype(B16)            # (4,128,128)
    c["mlp2"] = np.ascontiguousarray(np.concatenate([
        np.concatenate([c2r, c2i], axis=2),
        np.concatenate([-c2i, c2r], axis=2)], axis=1)).astype(B16)  # (4,128,128)
    b1r = W["cb1"][0].reshape(NB, 64, 1).astype(np.float32)
    b1i = W["cb1"][1].reshape(NB, 64, 1).astype(np.float32)
    c["b1s"] = np.ascontiguousarray(np.concatenate([b1r, b1i], axis=1))  # (4,128,1)
    b2r = W["cb2"][0].reshape(NB, 64, 1).astype(np.float32)
    b2i = W["cb2"][1].reshape(NB, 64, 1).astype(np.float32)
    c["b2p"] = np.ascontiguousarray(
        np.concatenate([b2r, b2i], axis=1) - LAM)                  # (4,128,1)
    c["b2n"] = np.ascontiguousarray(
        np.concatenate([-b2r, -b2i], axis=1) - LAM)                # (4,128,1)

    cs = np.zeros((32, 16), np.float32)
    for r in range(16):
        cs[r, :] = np.cos(2 * np.pi * a_i * r / 16.0)
        cs[16 + r, :] = -np.sin(2 * np.pi * a_i * r / 16.0)
    c["cs16"] = cs.astype(B16)
    return c


def _core_consts(W, core):
    b, j = core // 4, core % 4
    c = {}
    c["xb"] = np.ascontiguousarray(W["x"][b]).astype(np.float32)      # (2048,256)
    rb = DT_RANK + SLOC * j
    c["w_b_t"] = np.ascontiguousarray(
        W["x_proj_w"][rb:rb + SLOC].T.reshape(NDT, 128, SLOC)).astype(F16)
    rc = DT_RANK + D_STATE + SLOC * j
    c["w_c_t"] = np.ascontiguousarray(
        W["x_proj_w"][rc:rc + SLOC].T.reshape(NDT, 128, SLOC)).astype(F16)
    sv = -(np.arange(SLOC, dtype=np.float32) + 1.0 + SLOC * j)
    c["neg_s"] = np.ascontiguousarray(
        np.broadcast_to(sv[None, :], (128, SLOC))).astype(np.float32)
    # E6 block-iDFT combos for this core's output block j:
    # coef = 0.5*exp(2 pi i j kf / 4); z3r = sum_kf zr^T P + zi^T Q;
    # z3i = sum_kf zi^T P + zr^T R with P=Re(coef) I, Q=-Im(coef) I, R=Im I.
    eye = np.eye(64, dtype=np.float64)
    compR = np.zeros((NB, 128, 64), np.float64)
    compI = np.zeros((NB, 128, 64), np.float64)
    for kf in range(NB):
        coef = 0.5 * np.exp(2j * np.pi * j * kf / 4.0)
        P = np.round(coef.real, 6) * eye
        Q = -np.round(coef.imag, 6) * eye
        R = np.round(coef.imag, 6) * eye
        compR[kf] = np.concatenate([P, Q], axis=0)   # rows: [zr; zi]
        compI[kf] = np.concatenate([R, P], axis=0)
    c["compR"] = np.ascontiguousarray(compR).astype(B16)
    c["compI"] = np.ascontiguousarray(compI).astype(B16)
    return c


# --------------------------------------------------------------------------
# program builder
# --------------------------------------------------------------------------

def _split_multiwait_drains(nc):
    """This walrus build rejects instructions carrying >1 semaphore wait; move
    extra waits onto preceding single-wait Drain instructions (same engine)."""
    for f in nc.m.functions:
        for blk in f.blocks:
            new_list, changed = [], False
            for inst in blk.instructions:
                si = getattr(inst, "sync_info", None)
                if si is not None and si.on_wait and len(si.on_wait) > 1:
                    waits = list(si.on_wait)
                    for i, w in enumerate(waits[:-1]):
                        new_list.append(mybir.InstDrain(
                            name=f"{inst.name}_w{i}", engine=inst.engine,
                            ins=[], outs=[],
                            sync_info=mybir.SyncInfo(on_wait=[w], on_update=[]),
                            is_reset_sema=False))
                    inst.sync_info.on_wait = [waits[-1]]
                    changed = True
                new_list.append(inst)
            if changed:
                blk.instructions = new_list


def _ln_tiles(nc, x_tiles, w_bc, b_bc, out_tiles, scr, eps_ap, odt):
    """LayerNorm over the 256-wide free dim; out/w/b in 16-bit dtype odt."""
    sfx = str(odt)
    for ti in range(NTT):
        xt = x_tiles[ti]
        msum = scr.tile([128, 1], FP, name="ln_msum", tag="ln_msum")
        nc.vector.tensor_reduce(msum[:], xt[:], axis=mybir.AxisListType.X, op=ALU.add)
        mean = scr.tile([128, 1], FP, name="ln_mean", tag="ln_mean")
        nc.vector.tensor_scalar_mul(mean[:], msum[:], 1.0 / DIM)
        xm = scr.tile([128, DIM], odt, name="ln_xm", tag="ln_xm" + sfx)
        nc.vector.tensor_scalar(xm[:], xt[:], mean[:], None, ALU.subtract)
        sq = scr.tile([128, DIM], odt, name="ln_sq", tag="ln_sq" + sfx)
        vsum = scr.tile([128, 1], FP, name="ln_vsum", tag="ln_vsum")
        nc.scalar.activation(sq[:], xm[:], AF.Square, accum_out=vsum[:])
        std = scr.tile([128, 1], FP, name="ln_std", tag="ln_std")
        nc.scalar.activation(std[:], vsum[:], AF.Sqrt, bias=eps_ap[:], scale=1.0 / DIM)
        rstd = scr.tile([128, 1], FP, name="ln_rstd", tag="ln_rstd")
        nc.vector.reciprocal(rstd[:], std[:])
        ot = out_tiles[ti]
        nc.vector.scalar_tensor_tensor(ot[:], xm[:], rstd[:], w_bc[:],
                                       ALU.mult, ALU.mult)
        nc.vector.tensor_add(ot[:], ot[:], b_bc[:])


def build_program(num_cores=8, debug=False):
    nc = bass.Bass("TRN2", target_bir_lowering=False, debug=False,
                   num_devices=num_cores)

    I = {}
    FP16_IN = {"w_in_t", "w_dtraw_t", "w_b_t", "w_c_t", "w_dt_t", "w_out_t",
               "norm1_w_bc", "norm1_b_bc", "ident128", "sel16", "conv_diag"}
    BF16_IN = {"norm2_w_bc", "norm2_b_bc", "g_big", "fft_are", "fft_aim",
               "fft_naim", "fft_iare", "fft_iaim", "fft_niaim", "mlp1a",
               "mlp1b", "mlp2", "compR", "compI", "cs16"}
    for name, shape in [
        ("xb", (L, DIM)), ("w_in_t", (2, 128, 2 * D_INNER)),
        ("w_dtraw_t", (NDT, 128, DT_RANK)), ("w_b_t", (NDT, 128, SLOC)),
        ("w_c_t", (NDT, 128, SLOC)), ("w_dt_t", (DT_RANK, D_INNER)),
        ("w_out_t", (NDT, 128, DIM)),
        ("conv_diag", (NDT, D_CONV, 128, 128)),
        ("conv_b", (NDT, 128, 1)), ("dt_b", (NDT, 128, 1)), ("d_q", (NDT, 128, 1)),
        ("norm1_w_bc", (128, DIM)), ("norm1_b_bc", (128, DIM)),
        ("norm2_w_bc", (128, DIM)), ("norm2_b_bc", (128, DIM)),
        ("ident128", (128, 128)), ("neg_s", (128, SLOC)),
        ("sel16", (SLOC, SLOC * 128)),
        ("g_big", (64, 128)),
        ("fft_are", (16, 128, 128)), ("fft_aim", (16, 128, 128)),
        ("fft_naim", (16, 128, 128)), ("fft_iare", (16, 128, 128)),
        ("fft_iaim", (16, 128, 128)), ("fft_niaim", (16, 128, 128)),
        ("mlp1a", (NB, 128, 128)), ("mlp1b", (NB, 128, 128)),
        ("mlp2", (NB, 128, 128)),
        ("b1s", (NB, 128, 1)), ("b2p", (NB, 128, 1)), ("b2n", (NB, 128, 1)),
        ("compR", (NB, 128, 64)), ("compI", (NB, 128, 64)),
        ("cs16", (32, 16)),
    ]:
        dt_ = FP16 if name in FP16_IN else (BF16 if name in BF16_IN else FP)
        I[name] = nc.dram_tensor(name, list(shape), dt_, kind="ExternalInput")

    oblk = nc.dram_tensor("oblk", [L, BS], FP, kind="ExternalOutput")
    osumA = nc.dram_tensor("osumA", [L, DIM], BF16, kind="ExternalOutput")
    osumB = nc.dram_tensor("osumB", [L, DIM], BF16, kind="ExternalOutput")

    with tile.TileContext(nc, pool_alloc_mode="queue") as tc:
        _build_body(nc, tc, I, oblk, osumA, osumB)

    _split_multiwait_drains(nc)
    return nc


def _build_body(nc, tc, I, oblk, osumA, osumB):
    from contextlib import ExitStack
    ctx = ExitStack()
    with ctx:
        persist = ctx.enter_context(tc.tile_pool(name="persist", bufs=1))
        dram = ctx.enter_context(tc.tile_pool(name="dram", bufs=1, space="DRAM"))
        scr = ctx.enter_context(tc.tile_pool(name="scr", bufs=2))

        # ---------- persistent tiles ----------
        ln1w = persist.tile([128, DIM], FP16, name="ln1w", tag="ln1w")
        nc.sync.dma_start(ln1w[:], I["norm1_w_bc"][:])
        ln1b = persist.tile([128, DIM], FP16, name="ln1b", tag="ln1b")
        nc.sync.dma_start(ln1b[:], I["norm1_b_bc"][:])
        neg_s = persist.tile([128, SLOC], FP, name="neg_s", tag="neg_s")
        nc.sync.dma_start(neg_s[:], I["neg_s"][:])
        sel16 = persist.tile([SLOC, SLOC * 128], FP16, name="sel16", tag="sel16")
        nc.sync.dma_start(sel16[:], I["sel16"][:])
        bt_sb = persist.tile([SLOC, L], FP16, name="bt", tag="bt")
        ct_sb = persist.tile([SLOC, L], FP16, name="ct", tag="ct")
        eps_t = persist.tile([128, 1], FP, name="eps_t", tag="eps_t")
        nc.vector.memset(eps_t[:], EPS)
        ones_c = persist.tile([128, 1], FP, name="ones_c", tag="ones_c")
        nc.vector.memset(ones_c[:], 1.0)



        xp_big = [persist.tile([128, 8 * DIM], FP, name=f"xpb{c}",
                               tag=f"xpb{c}") for c in range(2)]
        xp_tiles = [xp_big[ti // 8][:, (ti % 8) * DIM:(ti % 8 + 1) * DIM]
                    for ti in range(NTT)]

        o_bnA = dram.tile([L, DIM], BF16, name="o_bnA")
        o_bnB = [dram.tile([L // 2, DIM], BF16, name=f"o_bnB{c}")
                 for c in range(2)]

        # per-d-tile working set for the scan phase
        pmain_cm = tc.tile_pool(name="pmain", bufs=1)
        pmain = pmain_cm.__enter__()
        pxc_cm = tc.tile_pool(name="pxc", bufs=1)
        pxc = pxc_cm.__enter__()
        xc_t = [pxc.tile([128, L], FP16, name=f"xc{i}", tag=f"xc{i}")
                for i in range(NDT)]
        sz_t = [pmain.tile([128, L], FP16, name=f"sz{i}", tag=f"sz{i}")
                for i in range(NDT)]
        dt_t = [pmain.tile([128, L], FP16, name=f"dt{i}", tag=f"dt{i}")
                for i in range(NDT)]
        u_t = [pmain.tile([128, L], FP16, name=f"u{i}", tag=f"u{i}")
               for i in range(NDT)]
        y_t = [pmain.tile([128, L], FP16, name=f"y{i}", tag=f"y{i}")
               for i in range(NDT)]

        # ======== front-end: LN1, transpose, in_proj, conv ========
        fepsA_cm = tc.tile_pool(name="fepsA", bufs=2, space="PSUM")
        fepsA = fepsA_cm.__enter__()
        pt_cm = tc.tile_pool(name="pt", bufs=1)
        ptp = pt_cm.__enter__()
        xnT = [ptp.tile([128, L], FP16, name=f"xnT{i}", tag=f"xnT{i}")
               for i in range(2)]
        with tc.tile_pool(name="pln", bufs=1) as pln:
            x_tiles = [pln.tile([128, DIM], FP, name=f"x{i}", tag=f"x{i}")
                       for i in range(NTT)]
            for ti in range(NTT):
                nc.sync.dma_start(x_tiles[ti][:],
                                  I["xb"][ti * 128:(ti + 1) * 128, :])
            xn_tiles = [pln.tile([128, DIM], FP16, name=f"xn{i}", tag=f"xn{i}")
                        for i in range(NTT)]
            _ln_tiles(nc, x_tiles, ln1w, ln1b, xn_tiles, scr, eps_t, FP16)
            ident = pln.tile([128, 128], FP16, name="ident", tag="ident")
            nc.sync.dma_start(ident[:], I["ident128"][:])
            for ti in range(NTT):
                for cb in range(2):
                    pt = fepsA.tile([128, 128], FP16, name="tpose", tag="tpose")
                    nc.tensor.transpose(
                        pt[:], xn_tiles[ti][:, cb * 128:(cb + 1) * 128], ident[:])
                    nc.scalar.copy(xnT[cb][:, ti * 128:(ti + 1) * 128], pt[:])

        with tc.tile_pool(name="pip", bufs=1) as pip:
            w_in = [pip.tile([128, 2 * D_INNER], FP16, name=f"win{k}",
                             tag=f"win{k}") for k in range(2)]
            for k in range(2):
                nc.sync.dma_start(w_in[k][:], I["w_in_t"][k])
            xi_pad = [pip.tile([128, L + D_CONV - 1], FP16, name=f"xip{i}",
                               tag=f"xip{i}") for i in range(NDT)]
            for i in range(NDT):
                nc.vector.memset(xi_pad[i][:, 0:D_CONV - 1], 0.0)
            for m in range(8):
                for n in range(4):
                    ps = fepsA.tile([128, 512], FP, name="inproj", tag="inproj")
                    for k in range(2):
                        nc.tensor.matmul(ps[:], w_in[k][:, m * 128:(m + 1) * 128],
                                         xnT[k][:, n * 512:(n + 1) * 512],
                                         start=(k == 0), stop=(k == 1))
                    if m < NDT:
                        nc.vector.tensor_copy(
                            xi_pad[m][:, D_CONV - 1 + n * 512:
                                      D_CONV - 1 + (n + 1) * 512], ps[:])
                    else:
                        nc.scalar.activation(
                            sz_t[m - NDT][:, n * 512:(n + 1) * 512], ps[:],
                            AF.Silu)

            cwd = [[pip.tile([128, 128], FP16, name=f"cwd{i}_{k}",
                             tag=f"cwd{i}_{k}") for k in range(D_CONV)]
                   for i in range(NDT)]
            cb_ = [pip.tile([128, 1], FP, name=f"cb{i}", tag=f"cb{i}")
                   for i in range(NDT)]
            for i in range(NDT):
                for k in range(D_CONV):
                    nc.sync.dma_start(cwd[i][k][:], I["conv_diag"][i][k])
                nc.sync.dma_start(cb_[i][:], I["conv_b"][i])
            # depthwise conv on PE: 4 shifted diagonal matmuls per chunk
            with tc.tile_pool(name="cvps", bufs=2, space="PSUM") as cvps:
                for i in range(NDT):
                    for n in range(4):
                        ps = cvps.tile([128, 512], FP, name="cvp", tag="cvp")
                        for k in range(D_CONV):
                            nc.tensor.matmul(
                                ps[:], cwd[i][k][:],
                                xi_pad[i][:, k + n * 512:k + n * 512 + 512],
                                start=(k == 0), stop=(k == D_CONV - 1))
                        nc.scalar.activation(
                            xc_t[i][:, n * 512:(n + 1) * 512], ps[:],
                            AF.Silu, bias=cb_[i][:])
        pt_cm.__exit__(None, None, None)   # free xnT
        fepsA_cm.__exit__(None, None, None)

        # ======== x_proj, dt_proj, u, y-init ========
        fepsB_cm = tc.tile_pool(name="fepsB", bufs=2, space="PSUM")
        fepsB = fepsB_cm.__enter__()
        with tc.tile_pool(name="pfx", bufs=1) as pfx:
            wdtr = [pfx.tile([128, DT_RANK], FP16, name=f"wdtr{k}",
                             tag=f"wdtr{k}") for k in range(NDT)]
            wbt = [pfx.tile([128, SLOC], FP16, name=f"wbt{k}", tag=f"wbt{k}")
                   for k in range(NDT)]
            wct = [pfx.tile([128, SLOC], FP16, name=f"wct{k}", tag=f"wct{k}")
                   for k in range(NDT)]
            for k in range(NDT):
                nc.sync.dma_start(wdtr[k][:], I["w_dtraw_t"][k])
                nc.sync.dma_start(wbt[k][:], I["w_b_t"][k])
                nc.sync.dma_start(wct[k][:], I["w_c_t"][k])
            dtraw = pfx.tile([DT_RANK, L], FP16, name="dtraw", tag="dtraw")
            for n in range(4):
                psd = fepsB.tile([DT_RANK, 512], FP, name="xprojD", tag="xprojD")
                psb = fepsB.tile([DT_RANK, 512], FP, name="xprojB", tag="xprojB")
                psc = fepsB.tile([DT_RANK, 512], FP, name="xprojC", tag="xprojC")
                for k in range(NDT):
                    xck = xc_t[k][:, n * 512:(n + 1) * 512]
                    nc.tensor.matmul(psd[:], wdtr[k][:], xck,
                                     start=(k == 0), stop=(k == NDT - 1))
                    nc.tensor.matmul(psb[:], wbt[k][:], xck,
                                     start=(k == 0), stop=(k == NDT - 1))
                    nc.tensor.matmul(psc[:], wct[k][:], xck,
                                     start=(k == 0), stop=(k == NDT - 1))
                nc.scalar.copy(dtraw[:, n * 512:(n + 1) * 512], psd[:])
                nc.scalar.copy(bt_sb[:, n * 512:(n + 1) * 512], psb[:])
                nc.scalar.copy(ct_sb[:, n * 512:(n + 1) * 512], psc[:])

            wdt = pfx.tile([DT_RANK, D_INNER], FP16, name="wdt", tag="wdt")
            nc.sync.dma_start(wdt[:], I["w_dt_t"][:])
            dtb = [pfx.tile([128, 1], FP, name=f"dtb{i}", tag=f"dtb{i}")
                   for i in range(NDT)]
            for i in range(NDT):
                nc.sync.dma_start(dtb[i][:], I["dt_b"][i])
            for i in range(NDT):
                for n in range(4):
                    ps = fepsB.tile([128, 512], FP, name="dtproj", tag="dtproj")
                    nc.tensor.matmul(ps[:], wdt[:, i * 128:(i + 1) * 128],
                                     dtraw[:, n * 512:(n + 1) * 512],
                                     start=True, stop=True)
                    # softplus: dt = ln(1 + exp(raw + b))
                    t1 = pfx.tile([128, 512], FP, name="sp_t1", tag="sp_t1",
                                  bufs=2)
                    nc.scalar.activation(t1[:], ps[:], AF.Exp, bias=dtb[i][:])
                    nc.scalar.activation(dt_t[i][:, n * 512:(n + 1) * 512],
                                         t1[:], AF.Ln, bias=ones_c[:])

            dq = [pfx.tile([128, 1], FP, name=f"dq{i}", tag=f"dq{i}")
                  for i in range(NDT)]
            for i in range(NDT):
                nc.sync.dma_start(dq[i][:], I["d_q"][i])
            for i in range(NDT):
                nc.vector.tensor_mul(u_t[i][:], dt_t[i][:], xc_t[i][:])
                nc.vector.tensor_scalar(y_t[i][:], xc_t[i][:], dq[i][:],
                                        None, ALU.mult)
        fepsB_cm.__exit__(None, None, None)
        pxc_cm.__exit__(None, None, None)   # free xc
        py2_cm = tc.tile_pool(name="py2", bufs=1)
        py2 = py2_cm.__enter__()
        y2_t = [py2.tile([128, L], FP16, name=f"y2_{i}", tag=f"y2_{i}")
                for i in range(NDT)]

        # FFT-phase constants: issued here so the DMAs hide under the scan
        ln2w = persist.tile([128, DIM], BF16, name="ln2w", tag="ln2w")
        nc.sync.dma_start(ln2w[:], I["norm2_w_bc"][:])
        ln2b = persist.tile([128, DIM], BF16, name="ln2b", tag="ln2b")
        nc.sync.dma_start(ln2b[:], I["norm2_b_bc"][:])

        # resident constants
        gbig = persist.tile([64, 128], BF16, name="gbig", tag="gbig")
        nc.sync.dma_start(gbig[:], I["g_big"][:])
        fare = persist.tile([128, 16 * 128], BF16, name="fare", tag="fare")
        faim = persist.tile([128, 16 * 128], BF16, name="faim", tag="faim")
        fnaim = persist.tile([128, 16 * 128], BF16, name="fnaim", tag="fnaim")
        fiare = persist.tile([128, 16 * 128], BF16, name="fiare", tag="fiare")
        fiaim = persist.tile([128, 16 * 128], BF16, name="fiaim", tag="fiaim")
        fniaim = persist.tile([128, 16 * 128], BF16, name="fniaim", tag="fniaim")
        for r in range(16):
            sl = slice(r * 128, (r + 1) * 128)
            nc.sync.dma_start(fare[:, sl], I["fft_are"][r])
            nc.sync.dma_start(faim[:, sl], I["fft_aim"][r])
            nc.sync.dma_start(fnaim[:, sl], I["fft_naim"][r])
            nc.sync.dma_start(fiare[:, sl], I["fft_iare"][r])
            nc.sync.dma_start(fiaim[:, sl], I["fft_iaim"][r])
            nc.sync.dma_start(fniaim[:, sl], I["fft_niaim"][r])
        m1a = [persist.tile([128, 128], BF16, name=f"m1a{k}", tag=f"m1a{k}")
               for k in range(NB)]
        m1b = [persist.tile([128, 128], BF16, name=f"m1b{k}", tag=f"m1b{k}")
               for k in range(NB)]
        m2 = [persist.tile([128, 128], BF16, name=f"m2_{k}", tag=f"m2_{k}")
              for k in range(NB)]
        b1s = [persist.tile([128, 1], FP, name=f"b1s{k}", tag=f"b1s{k}")
               for k in range(NB)]
        b2p = [persist.tile([128, 1], FP, name=f"b2p{k}", tag=f"b2p{k}")
               for k in range(NB)]
        b2n = [persist.tile([128, 1], FP, name=f"b2n{k}", tag=f"b2n{k}")
               for k in range(NB)]
        cmpR = [persist.tile([128, 64], BF16, name=f"cmR{k}", tag=f"cmR{k}")
                for k in range(NB)]
        cmpI = [persist.tile([128, 64], BF16, name=f"cmI{k}", tag=f"cmI{k}")
                for k in range(NB)]
        for k in range(NB):
            nc.sync.dma_start(m1a[k][:], I["mlp1a"][k])
            nc.sync.dma_start(m1b[k][:], I["mlp1b"][k])
            nc.sync.dma_start(m2[k][:], I["mlp2"][k])
            nc.sync.dma_start(b1s[k][:], I["b1s"][k])
            nc.sync.dma_start(b2p[k][:], I["b2p"][k])
            nc.sync.dma_start(b2n[k][:], I["b2n"][k])
            nc.sync.dma_start(cmpR[k][:], I["compR"][k])
            nc.sync.dma_start(cmpI[k][:], I["compI"][k])
        cs16 = persist.tile([32, 16], BF16, name="cs16", tag="cs16")
        nc.sync.dma_start(cs16[:], I["cs16"][:])

        # ================= scan phase (si-split AllReduce) =================
        wout = [persist.tile([128, DIM], FP16, name=f"wout{k}", tag=f"wout{k}")
                for k in range(NDT)]
        for k in range(NDT):
            nc.sync.dma_start(wout[k][:], I["w_out_t"][k])
        o_sA = dram.tile([L, DIM], BF16, name="o_sA")
        o_sB = [dram.tile([L // 2, DIM], BF16, name=f"o_sB{c}")
                for c in range(2)]

        with tc.tile_pool(name="scn", bufs=2) as sp, \
             tc.tile_pool(name="scn1", bufs=1) as sp1, \
             tc.tile_pool(name="scnps", bufs=2, space="PSUM") as sps, \
             tc.tile_pool(name="arps", bufs=2, space="PSUM") as arps:
            for si in range(SLOC):
                bbc16 = sp.tile([128, L], FP16, name="bbc16", tag="bbc16")
                cbc16 = sp.tile([128, L], FP16, name="cbc16", tag="cbc16")
                selw = sel16[:, si * 128:(si + 1) * 128]
                for n in range(4):
                    csl = slice(n * 512, (n + 1) * 512)
                    bps = sps.tile([128, 512], FP, name="bps", tag="bps")
                    nc.tensor.matmul(bps[:], selw, bt_sb[:, csl],
                                     start=True, stop=True)
                    nc.scalar.copy(bbc16[:, csl], bps[:])
                    cps = sps.tile([128, 512], FP, name="cps", tag="cps")
                    nc.tensor.matmul(cps[:], selw, ct_sb[:, csl],
                                     start=True, stop=True)
                    nc.scalar.copy(cbc16[:, csl], cps[:])
                # group A: si 0..7 (y_t, includes D-term init); group B:
                # si 8..15 (y2_t).
                yt = y_t if si < 8 else y2_t
                for i in range(NDT):
                    dA = sp.tile([128, L], FP16, name="dA", tag="dA")
                    nc.scalar.activation(dA[:], dt_t[i][:], AF.Exp,
                                         scale=neg_s[:, si:si + 1])
                    dBu = sp1.tile([128, L], FP16, name="dBu", tag="dBu")
                    nc.vector.tensor_mul(dBu[:], u_t[i][:], bbc16[:])
                    h = sp1.tile([128, L], FP16, name="h", tag="h")
                    nc.vector.tensor_tensor_scan(h[:], dA[:], dBu[:], 0.0,
                                                 ALU.mult, ALU.add)
                    if si == 8:
                        nc.vector.tensor_mul(yt[i][:], h[:], cbc16[:])
                    else:
                        w = sp1.tile([128, L], FP16, name="w", tag="w")
                        nc.vector.tensor_mul(w[:], h[:], cbc16[:])
                        nc.vector.tensor_add(yt[i][:], yt[i][:], w[:])
                if si == 7:
                    # group-A gate + out_proj + AllReduce; the collective
                    # runs during the si=8..15 scanning.
                    for i in range(NDT):
                        nc.vector.tensor_mul(y_t[i][:], y_t[i][:],
                                             sz_t[i][:])
                    for ti in range(NTT):
                        ps = arps.tile([128, DIM], FP, name="oprojA",
                                       tag="oprojA")
                        for k in range(NDT):
                            nc.tensor.matmul(
                                ps[:], y_t[k][:, ti * 128:(ti + 1) * 128],
                                wout[k][:], start=(k == 0),
                                stop=(k == NDT - 1))
                        ot = scr.tile([128, DIM], BF16, name="oevA",
                                      tag="oevA")
                        nc.scalar.copy(ot[:], ps[:])
                        nc.sync.dma_start(
                            o_bnA[ti * 128:(ti + 1) * 128, :], ot[:])
                    nc.gpsimd.collective_compute(
                        "AllReduce", ALU.add,
                        replica_groups=[[0, 1, 2, 3], [4, 5, 6, 7]],
                        ins=[o_bnA.opt()], outs=[o_sA.opt()])
                    nc.sync.dma_start(osumA[:], o_sA[:])

        py2_cm.__exit__(None, None, None)   # free y2 (group B projected)

        # ==== group-B gate + out_proj + per-half AllReduce; xp assembly ====
        with tc.tile_pool(name="postps", bufs=2, space="PSUM") as pops, \
             tc.tile_pool(name="pxa", bufs=1) as pxa:
            for i in range(NDT):
                nc.vector.tensor_mul(y2_t[i][:], y2_t[i][:], sz_t[i][:])
            for c in range(2):
                hsl = slice(c * 1024, (c + 1) * 1024)
                for tj in range(8):
                    ti = c * 8 + tj
                    ps = pops.tile([128, DIM], FP, name="oproj", tag="oproj")
                    for k in range(NDT):
                        nc.tensor.matmul(
                            ps[:], y2_t[k][:, ti * 128:(ti + 1) * 128],
                            wout[k][:], start=(k == 0), stop=(k == NDT - 1))
                    ot = scr.tile([128, DIM], BF16, name="oev", tag="oev")
                    nc.scalar.copy(ot[:], ps[:])
                    nc.sync.dma_start(o_bnB[c][tj * 128:(tj + 1) * 128, :],
                                      ot[:])
                nc.gpsimd.collective_compute(
                    "AllReduce", ALU.add,
                    replica_groups=[[0, 1, 2, 3], [4, 5, 6, 7]],
                    ins=[o_bnB[c].opt()], outs=[o_sB[c].opt()])
                nc.sync.dma_start(osumB[hsl, :], o_sB[c][:])
                # xp half = x + osumA + osumB (bulk rearranged-AP loads)
                ld = pxa.tile([128, 8 * DIM], BF16, name="ld", tag="ld")
                nc.sync.dma_start(
                    ld[:].rearrange("p (t d) -> p t d", t=8),
                    o_sA[hsl, :].rearrange("(t p) d -> p t d", p=128))
                t1 = pxa.tile([128, 8 * DIM], FP, name="t1", tag="t1")
                nc.scalar.copy(t1[:], ld[:])
                ld2 = pxa.tile([128, 8 * DIM], BF16, name="ld2", tag="ld2")
                nc.sync.dma_start(
                    ld2[:].rearrange("p (t d) -> p t d", t=8),
                    o_sB[c][:].rearrange("(t p) d -> p t d", p=128))
                t2 = pxa.tile([128, 8 * DIM], FP, name="t2", tag="t2")
                nc.scalar.copy(t2[:], ld2[:])
                nc.vector.tensor_add(t1[:], t1[:], t2[:])
                nc.sync.dma_start(
                    t2[:].rearrange("p (t d) -> p t d", t=8),
                    I["xb"][hsl, :].rearrange("(t p) d -> p t d", p=128))
                nc.vector.tensor_add(xp_big[c][:], t2[:], t1[:])
        pmain_cm.__exit__(None, None, None)   # free sz/dt/u/y

        # ================= LN2 + EinFFT (bf16, SBUF-resident) =================
        with tc.tile_pool(name="fft", bufs=1) as ff, \
             tc.tile_pool(name="fftsc", bufs=2) as fsc:

            pack = ff.tile([64, 128 * BS], BF16, name="pack", tag="pack")
            gfull = ff.tile([128, 128 * BS], BF16, name="gfull", tag="gfull")
            ftre = [ff.tile([128, L], BF16, name=f"ftre{u}", tag=f"ftre{u}")
                    for u in range(2)]
            ftim = [ff.tile([128, L], BF16, name=f"ftim{u}", tag=f"ftim{u}")
                    for u in range(2)]
            z2st = [ff.tile([128, L], BF16, name=f"z2st{k}", tag=f"z2st{k}")
                    for k in range(NB)]
            wpack = ff.tile([32, 128 * BS], BF16, name="wpack", tag="wpack")

            fpsG_cm = tc.tile_pool(name="fpsG", bufs=2, space="PSUM")
            fpsG = fpsG_cm.__enter__()
            # --- LN2 + pack (token-transpose via SBUF->SBUF DMA)
            with tc.tile_pool(name="fftln", bufs=1) as fln:
                xn2 = [fln.tile([128, DIM], BF16, name=f"xn2_{i}",
                                tag=f"xn2_{i}") for i in range(NTT)]
                _ln_tiles(nc, xp_tiles, ln2w, ln2b, xn2, scr, eps_t, BF16)
                for a in range(16):
                    for k in range(NB):
                        row = 4 * a + k
                        eng = (nc.sync, nc.scalar)[row % 2]
                        eng.dma_start(pack[row:row + 1, :],
                                      xn2[a][:, 64 * k:64 * (k + 1)])

                # --- G: (a,k) -> (kf,r) mixing; out rows = [re(64); im(64)]
                gfull3 = gfull[:].rearrange("m (d s) -> m d s", d=64)
                for n in range(16):
                    sl = slice(n * 512, (n + 1) * 512)
                    psg = fpsG.tile([128, 512], FP, name="gps", tag="gps")
                    nc.tensor.matmul(psg[:], gbig[:], pack[:, sl],
                                     start=True, stop=True)
                    nc.vector.tensor_copy(
                        gfull3[:, :, 8 * n:8 * (n + 1)],
                        psg[:].rearrange("m (s d) -> m d s", s=8))

            # --- E4 input: XBAR DMA transpose (one instruction per half)
            # packT_*[s, d, (16kf+r)] = G[(kf,r), (s,d)]
            packT_re = ff.tile([128, 4096], BF16, name="packT_re",
                               tag="packT_re")
            packT_im = ff.tile([128, 4096], BF16, name="packT_im",
                               tag="packT_im")
            nc.sync.dma_start_transpose(
                packT_re[:].rearrange("p (e c) -> p e c", e=64),
                gfull[0:64, :])
            nc.sync.dma_start_transpose(
                packT_im[:].rearrange("p (e c) -> p e c", e=64),
                gfull[64:128, :])
            vre = packT_re[:].rearrange("p (d c) -> p c d", d=64)
            vim = packT_im[:].rearrange("p (d c) -> p c d", d=64)
            gtr_r = [ff.tile([128, 256], BF16, name=f"gtr{r}", tag=f"gtr{r}")
                     for r in range(16)]
            gti_r = [ff.tile([128, 256], BF16, name=f"gti{r}", tag=f"gti{r}")
                     for r in range(16)]
            for r in range(16):
                nc.vector.tensor_copy(gtr_r[r][:], vre[:, r::16, :])
                nc.vector.tensor_copy(gti_r[r][:], vim[:, r::16, :])

            # --- E4 / E5 / E6 / E7 interleaved by r-group so the PE,
            # Scalar, and Vector queues pipeline across stages. Every
            # accumulation group gets its own PSUM tile (banks must not be
            # shared between concurrent accumulation groups).
            fps5_cm = tc.tile_pool(name="fps5", bufs=1, space="PSUM")
            fps5 = fps5_cm.__enter__()
            fps6_cm = tc.tile_pool(name="fps6", bufs=1, space="PSUM")
            fps6 = fps6_cm.__enter__()
            for n in range(4):
                for r in range(4 * n, 4 * n + 4):
                    rsl = slice(r * 128, (r + 1) * 128)
                    gtr, gti = gtr_r[r], gti_r[r]
                    for u in range(2):
                        lsl = slice(u * 128, (u + 1) * 128)
                        pre = fpsG.tile([128, 128], FP, name="e4ps",
                                        tag="e4ps")
                        nc.tensor.matmul(pre[:], gtr[:, lsl], fare[:, rsl],
                                         start=True, stop=False)
                        nc.tensor.matmul(pre[:], gti[:, lsl], fnaim[:, rsl],
                                         start=False, stop=True)
                        nc.vector.tensor_copy(ftre[u][:, rsl], pre[:])
                        pim = fpsG.tile([128, 128], FP, name="e4ps",
                                        tag="e4ps")
                        nc.tensor.matmul(pim[:], gtr[:, lsl], faim[:, rsl],
                                         start=True, stop=False)
                        nc.tensor.matmul(pim[:], gti[:, lsl], fare[:, rsl],
                                         start=False, stop=True)
                        nc.vector.tensor_copy(ftim[u][:, rsl], pim[:])
                sl = slice(n * 512, (n + 1) * 512)
                for kf in range(NB):
                    u, lo = kf // 2, 64 * (kf % 2)
                    fre = ftre[u][lo:lo + 64, :]
                    fim = ftim[u][lo:lo + 64, :]
                    l1 = fps5.tile([128, 512], FP, name="mlp1", tag="mlp")
                    nc.tensor.matmul(l1[:], m1a[kf][lo:lo + 64, :],
                                     fre[:, sl], start=True, stop=False)
                    nc.tensor.matmul(l1[:], m1b[kf][lo:lo + 64, :],
                                     fim[:, sl], start=False, stop=True)
                    r1i1 = fsc.tile([128, 512], BF16, name="r1i1",
                                    tag="r1i1")
                    nc.scalar.activation(r1i1[:], l1[:], AF.Relu,
                                         bias=b1s[kf][:])
                    l2 = fps5.tile([128, 512], FP, name="mlp2", tag="mlp")
                    nc.tensor.matmul(l2[:], m2[kf][:], r1i1[:],
                                     start=True, stop=True)
                    shp = fsc.tile([128, 512], BF16, name="shp", tag="shp")
                    nc.scalar.activation(shp[:], l2[:], AF.Relu,
                                         bias=b2p[kf][:])
                    shn = fsc.tile([128, 512], BF16, name="shn", tag="shn")
                    nc.scalar.activation(shn[:], l2[:], AF.Relu,
                                         bias=b2n[kf][:], scale=-1.0)
                    nc.vector.tensor_sub(z2st[kf][:, sl], shp[:], shn[:])
                for r in range(4 * n, 4 * n + 4):
                    rsl = slice(r * 128, (r + 1) * 128)
                    z3r = fps6.tile([128, BS], FP, name="e6ps", tag="e6ps")
                    z3i = fps6.tile([128, BS], FP, name="e6ps2",
                                    tag="e6ps2")
                    for kf in range(NB):
                        nc.tensor.matmul(z3r[:], z2st[kf][:, rsl],
                                         cmpR[kf][:], start=(kf == 0),
                                         stop=(kf == NB - 1))
                        nc.tensor.matmul(z3i[:], z2st[kf][:, rsl],
                                         cmpI[kf][:], start=(kf == 0),
                                         stop=(kf == NB - 1))
                    z3r_sb = fsc.tile([128, BS], BF16, name="z3sb",
                                      tag="z3sb")
                    nc.scalar.copy(z3r_sb[:], z3r[:])
                    z3i_sb = fsc.tile([128, BS], BF16, name="z3sb2",
                                      tag="z3sb2")
                    nc.scalar.copy(z3i_sb[:], z3i[:])
                    pw = fps6.tile([128, BS], FP, name="e7ps", tag="e7ps")
                    nc.tensor.matmul(pw[:], fiare[:, rsl], z3r_sb[:],
                                     start=True, stop=False)
                    nc.tensor.matmul(pw[:], fniaim[:, rsl], z3i_sb[:],
                                     start=False, stop=True)
                    wev = fsc.tile([128, BS], BF16, name="wev", tag="wev")
                    nc.scalar.copy(wev[:], pw[:])
                    nc.sync.dma_start(wpack[r:r + 1, :], wev[:])
                    pw2 = fps6.tile([128, BS], FP, name="e7ps2",
                                    tag="e7ps")
                    nc.tensor.matmul(pw2[:], fiaim[:, rsl], z3r_sb[:],
                                     start=True, stop=False)
                    nc.tensor.matmul(pw2[:], fiare[:, rsl], z3i_sb[:],
                                     start=False, stop=True)
                    wev2 = fsc.tile([128, BS], BF16, name="wev", tag="wev")
                    nc.scalar.copy(wev2[:], pw2[:])
                    nc.sync.dma_start(wpack[16 + r:16 + r + 1, :], wev2[:])
            fps6_cm.__exit__(None, None, None)
            fps5_cm.__exit__(None, None, None)
            fps8_cm = tc.tile_pool(name="fps8", bufs=2, space="PSUM")
            fps8 = fps8_cm.__enter__()
            # --- E8: final recombination over r; stream chunks to oblk
            oblk_v = oblk[:].rearrange("(a p) d -> a p d", a=16)
            for n in range(16):
                sl = slice(n * 512, (n + 1) * 512)
                ps = fps8.tile([16, 512], FP, name="eops", tag="eops")
                nc.tensor.matmul(ps[:], cs16[:], wpack[:, sl],
                                 start=True, stop=True)
                ev = fsc.tile([16, 512], FP, name="eoev", tag="eoev")
                nc.scalar.copy(ev[:], ps[:])
                nc.sync.dma_start(oblk_v[:, 8 * n:8 * (n + 1), :], ev[:])
            fps8_cm.__exit__(None, None, None)


# --------------------------------------------------------------------------
# host entry point
# --------------------------------------------------------------------------

_CACHE = {}


def _get_program(debug=False):
    key = ("prog", debug)
    if key not in _CACHE:
        _CACHE[key] = build_program(num_cores=8, debug=debug)
    return _CACHE[key]


def run(inputs, debug=False, trace=False):
    W = {k: np.asarray(v) for k, v in inputs.items() if k not in ("H", "W")}
    shared = _shared_consts(W)
    in_maps = []
    for core in range(8):
        m = dict(shared)
        m.update(_core_consts(W, core))
        in_maps.append(m)
    nc = _get_program(debug=debug)
    res = run_bass_kernel_spmd(nc, in_maps, core_ids=list(range(8)), trace=trace)
    x = np.asarray(W["x"], np.float32)
    out = np.empty((B_SZ, L, DIM), np.float32)
    for core in range(8):
        b, j = core // 4, core % 4
        sl = slice(BS * j, BS * (j + 1))
        r = res.results[core]
        osum = (r["osumA"].astype(np.float32) + r["osumB"].astype(np.float32))
        out[b, :, sl] = x[b][:, sl] + osum[:, sl] + r["oblk"]
    return out, res


def kernel(**inputs):
    out, _ = run(inputs)
    return out


# revision 45
# speedup vs baseline: 1.1969x; 1.1969x over previous
"""Trainium2 Bass kernel for nn_Block_mamba (Mamba block + EinFFT block).

Sharding over 8 NeuronCores: core c -> batch b = c // 4, state-block j = c % 4.
Each core runs the full front-end for its batch (LN1, in_proj, causal depthwise
conv, x_proj, dt_proj) in fp16/bf16 matmul precision, scans its 16 SSM states
over all 512 channels with the DVE tensor_tensor_scan instruction (fp16
operands), forms the partial out-projection, AllReduces it within the 4-core
batch group in 4 token chunks (overlapped with compute), then computes LN2 +
its EinFFT output block entirely in SBUF with bf16 matmuls (fft2 mixes the
token axis and the 4-block axis; the length-4 block DFT is folded into
constant matrices; the 2048-point token DFT is factorized as 16 x 128 with
twiddles folded into per-r constant matrices).

Self-contained: hardcodes all shapes; host-side code is numpy only.
"""
import os
import sys
import numpy as np

for _p in ("/opt/trn_rl_repo", os.path.expanduser("~/.axon_site/_ro/trn_rl_repo")):
    if os.path.isdir(_p) and _p not in sys.path:
        sys.path.insert(0, _p)

import ml_dtypes
import concourse.bass as bass
import concourse.tile as tile
from concourse import mybir
from concourse.bass_utils import run_bass_kernel_spmd

FP = mybir.dt.float32
FP16 = mybir.dt.float16
BF16 = mybir.dt.bfloat16
ALU = mybir.AluOpType
AF = mybir.ActivationFunctionType

DIM = 256
D_STATE = 64
D_CONV = 4
D_INNER = 512
DT_RANK = 16
NB = 4
BS = 64
B_SZ = 2
L = 2048
LAM = 0.01
NDT = D_INNER // 128          # 4 d-tiles
NTT = L // 128                # 16 t-tiles
SLOC = 16                     # states per core
EPS = 1e-5
NAR = 4                       # AllReduce chunks (tokens)

F16 = np.float16
B16 = ml_dtypes.bfloat16


# --------------------------------------------------------------------------
# host-side constants
# --------------------------------------------------------------------------

def _shared_consts(W):
    c = {}
    c["w_in_t"] = np.ascontiguousarray(
        W["in_proj_w"].T.reshape(2, 128, 2 * D_INNER)).astype(F16)
    c["w_dtraw_t"] = np.ascontiguousarray(
        W["x_proj_w"][:DT_RANK].T.reshape(NDT, 128, DT_RANK)).astype(F16)
    c["w_dt_t"] = np.ascontiguousarray(W["dt_proj_w"].T).astype(F16)  # (16,512)
    c["w_out_t"] = np.ascontiguousarray(
        W["out_proj_w"].T.reshape(NDT, 128, DIM)).astype(F16)
    cwr = W["conv_w"][:, 0, :].reshape(NDT, 128, D_CONV)
    cdiag = np.zeros((NDT, D_CONV, 128, 128), np.float32)
    for i in range(NDT):
        for k in range(D_CONV):
            np.fill_diagonal(cdiag[i, k], cwr[i, :, k])
    c["conv_diag"] = cdiag.astype(F16)
    c["conv_b"] = np.ascontiguousarray(
        W["conv_b"].reshape(NDT, 128, 1)).astype(np.float32)
    c["dt_b"] = np.ascontiguousarray(
        W["dt_proj_b"].reshape(NDT, 128, 1)).astype(np.float32)
    c["d_q"] = np.ascontiguousarray(
        (W["D"] / 4.0).reshape(NDT, 128, 1)).astype(np.float32)  # /4: AllReduce sums 4x
    c["norm1_w_bc"] = np.ascontiguousarray(
        np.broadcast_to(W["norm1_w"][None, :], (128, DIM))).astype(F16)
    c["norm1_b_bc"] = np.ascontiguousarray(
        np.broadcast_to(W["norm1_b"][None, :], (128, DIM))).astype(F16)
    c["norm2_w_bc"] = np.ascontiguousarray(
        np.broadcast_to(W["norm2_w"][None, :], (128, DIM))).astype(B16)
    c["norm2_b_bc"] = np.ascontiguousarray(
        np.broadcast_to(W["norm2_b"][None, :], (128, DIM))).astype(B16)
    c["ident128"] = np.eye(128, dtype=F16)
    sel = np.zeros((SLOC, SLOC, 128), np.float32)
    for si in range(SLOC):
        sel[si, si, :] = 1.0
    c["sel16"] = np.ascontiguousarray(
        sel.transpose(1, 0, 2).reshape(SLOC, SLOC * 128)).astype(F16)
    # sel16[k, si*128+m] = 1 iff k == si  -> lhsT slice (16,128) per si

    # ---- EinFFT constants (all bf16 matmul operands)
    a_i = np.arange(16)
    # G-stage lhsT[(4a+k), (16kf+r)] = e^{-2pi i a r/16} * 1/2 * e^{-2pi i kf k/4}
    Mc = np.zeros((64, 64), np.complex128)
    for a in range(16):
        for k in range(NB):
            for kf in range(NB):
                for r in range(16):
                    Mc[4 * a + k, 16 * kf + r] = 0.5 * np.exp(
                        -2j * np.pi * (a * r / 16.0 + kf * k / 4.0))
    c["g_big"] = np.ascontiguousarray(
        np.concatenate([Mc.real, Mc.imag], axis=1)).astype(B16)   # (64,128)

    p = np.arange(128)
    s = np.arange(128)
    are = np.zeros((16, 128, 128), np.float32)
    aim = np.zeros((16, 128, 128), np.float32)
    iare = np.zeros((16, 128, 128), np.float32)
    iaim = np.zeros((16, 128, 128), np.float32)
    for r in range(16):
        ph = np.outer(s, p) / 128.0 + p[None, :] * r / 2048.0      # [s,p]
        Ar = np.exp(-2j * np.pi * ph) / np.sqrt(L)                  # A_r[s,p]
        are[r] = Ar.real.T.astype(np.float32)                       # A_r^T [p,s]
        aim[r] = Ar.imag.T.astype(np.float32)
        iph = np.outer(p, s) / 128.0 + p[:, None] * r / 2048.0      # [p,s]
        IAr = np.exp(2j * np.pi * iph) / np.sqrt(L)                 # IA_r[p,s]
        iare[r] = IAr.real.T.astype(np.float32)                     # IA_r^T [s,p]
        iaim[r] = IAr.imag.T.astype(np.float32)
    c["fft_are"] = are.astype(B16)
    c["fft_aim"] = aim.astype(B16)
    c["fft_naim"] = (-aim).astype(B16)
    c["fft_iare"] = iare.astype(B16)
    c["fft_iaim"] = iaim.astype(B16)
    c["fft_niaim"] = (-iaim).astype(B16)

    c1r = W["cw1"][0].astype(np.float64)   # (4,64,64) [d,o]
    c1i = W["cw1"][1].astype(np.float64)
    c2r = W["cw2"][0].astype(np.float64)
    c2i = W["cw2"][1].astype(np.float64)
    # layer-1 lhsT, duplicated into both partition halves so the lhsT base
    # partition can match the rhs (ft tiles hold kf pairs stacked on
    # partitions; bass requires equal base partitions).
    l1a = np.concatenate([c1r, c1i], axis=2)                       # (4,64,128)
    l1b = np.concatenate([-c1i, c1r], axis=2)
    c["mlp1a"] = np.ascontiguousarray(
        np.concatenate([l1a, l1a], axis=1)).astype(B16)            # (4,128,128)
    c["mlp1b"] = np.ascontiguousarray(
        np.concatenate([l1b, l1b], axis=1)).astype(B16)            # (4,128,128)
    c["mlp2"] = np.ascontiguousarray(np.concatenate([
        np.concatenate([c2r, c2i], axis=2),
        np.concatenate([-c2i, c2r], axis=2)], axis=1)).astype(B16)  # (4,128,128)
    b1r = W["cb1"][0].reshape(NB, 64, 1).astype(np.float32)
    b1i = W["cb1"][1].reshape(NB, 64, 1).astype(np.float32)
    c["b1s"] = np.ascontiguousarray(np.concatenate([b1r, b1i], axis=1))  # (4,128,1)
    b2r = W["cb2"][0].reshape(NB, 64, 1).astype(np.float32)
    b2i = W["cb2"][1].reshape(NB, 64, 1).astype(np.float32)
    c["b2p"] = np.ascontiguousarray(
        np.concatenate([b2r, b2i], axis=1) - LAM)                  # (4,128,1)
    c["b2n"] = np.ascontiguousarray(
        np.concatenate([-b2r, -b2i], axis=1) - LAM)                # (4,128,1)

    cs = np.zeros((32, 16), np.float32)
    for r in range(16):
        cs[r, :] = np.cos(2 * np.pi * a_i * r / 16.0)
        cs[16 + r, :] = -np.sin(2 * np.pi * a_i * r / 16.0)
    c["cs16"] = cs.astype(B16)
    return c


def _core_consts(W, core):
    b, j = core // 4, core % 4
    c = {}
    c["xb"] = np.ascontiguousarray(W["x"][b]).astype(np.float32)      # (2048,256)
    rb = DT_RANK + SLOC * j
    c["w_b_t"] = np.ascontiguousarray(
        W["x_proj_w"][rb:rb + SLOC].T.reshape(NDT, 128, SLOC)).astype(F16)
    rc = DT_RANK + D_STATE + SLOC * j
    c["w_c_t"] = np.ascontiguousarray(
        W["x_proj_w"][rc:rc + SLOC].T.reshape(NDT, 128, SLOC)).astype(F16)
    sv = -(np.arange(SLOC, dtype=np.float32) + 1.0 + SLOC * j)
    c["neg_s"] = np.ascontiguousarray(
        np.broadcast_to(sv[None, :], (128, SLOC))).astype(np.float32)
    # E6 block-iDFT combos for this core's output block j:
    # coef = 0.5*exp(2 pi i j kf / 4); z3r = sum_kf zr^T P + zi^T Q;
    # z3i = sum_kf zi^T P + zr^T R with P=Re(coef) I, Q=-Im(coef) I, R=Im I.
    eye = np.eye(64, dtype=np.float64)
    compR = np.zeros((NB, 128, 64), np.float64)
    compI = np.zeros((NB, 128, 64), np.float64)
    for kf in range(NB):
        coef = 0.5 * np.exp(2j * np.pi * j * kf / 4.0)
        P = np.round(coef.real, 6) * eye
        Q = -np.round(coef.imag, 6) * eye
        R = np.round(coef.imag, 6) * eye
        compR[kf] = np.concatenate([P, Q], axis=0)   # rows: [zr; zi]
        compI[kf] = np.concatenate([R, P], axis=0)
    c["compR"] = np.ascontiguousarray(compR).astype(B16)
    c["compI"] = np.ascontiguousarray(compI).astype(B16)
    return c


# --------------------------------------------------------------------------
# program builder
# --------------------------------------------------------------------------

def _split_multiwait_drains(nc):
    """This walrus build rejects instructions carrying >1 semaphore wait; move
    extra waits onto preceding single-wait Drain instructions (same engine)."""
    for f in nc.m.functions:
        for blk in f.blocks:
            new_list, changed = [], False
            for inst in blk.instructions:
                si = getattr(inst, "sync_info", None)
                if si is not None and si.on_wait and len(si.on_wait) > 1:
                    waits = list(si.on_wait)
                    for i, w in enumerate(waits[:-1]):
                        new_list.append(mybir.InstDrain(
                            name=f"{inst.name}_w{i}", engine=inst.engine,
                            ins=[], outs=[],
                            sync_info=mybir.SyncInfo(on_wait=[w], on_update=[]),
                            is_reset_sema=False))
                    inst.sync_info.on_wait = [waits[-1]]
                    changed = True
                new_list.append(inst)
            if changed:
                blk.instructions = new_list


def _ln_tiles(nc, x_tiles, w_bc, b_bc, out_tiles, scr, eps_ap, odt):
    """LayerNorm over the 256-wide free dim; out/w/b in 16-bit dtype odt."""
    sfx = str(odt)
    for ti in range(NTT):
        xt = x_tiles[ti]
        msum = scr.tile([128, 1], FP, name="ln_msum", tag="ln_msum")
        nc.vector.tensor_reduce(msum[:], xt[:], axis=mybir.AxisListType.X, op=ALU.add)
        mean = scr.tile([128, 1], FP, name="ln_mean", tag="ln_mean")
        nc.vector.tensor_scalar_mul(mean[:], msum[:], 1.0 / DIM)
        xm = scr.tile([128, DIM], odt, name="ln_xm", tag="ln_xm" + sfx)
        nc.vector.tensor_scalar(xm[:], xt[:], mean[:], None, ALU.subtract)
        sq = scr.tile([128, DIM], odt, name="ln_sq", tag="ln_sq" + sfx)
        vsum = scr.tile([128, 1], FP, name="ln_vsum", tag="ln_vsum")
        nc.scalar.activation(sq[:], xm[:], AF.Square, accum_out=vsum[:])
        std = scr.tile([128, 1], FP, name="ln_std", tag="ln_std")
        nc.scalar.activation(std[:], vsum[:], AF.Sqrt, bias=eps_ap[:], scale=1.0 / DIM)
        rstd = scr.tile([128, 1], FP, name="ln_rstd", tag="ln_rstd")
        nc.vector.reciprocal(rstd[:], std[:])
        ot = out_tiles[ti]
        nc.vector.scalar_tensor_tensor(ot[:], xm[:], rstd[:], w_bc[:],
                                       ALU.mult, ALU.mult)
        nc.vector.tensor_add(ot[:], ot[:], b_bc[:])


def build_program(num_cores=8, debug=False):
    nc = bass.Bass("TRN2", target_bir_lowering=False, debug=False,
                   num_devices=num_cores)

    I = {}
    FP16_IN = {"w_in_t", "w_dtraw_t", "w_b_t", "w_c_t", "w_dt_t", "w_out_t",
               "norm1_w_bc", "norm1_b_bc", "ident128", "sel16", "conv_diag"}
    BF16_IN = {"norm2_w_bc", "norm2_b_bc", "g_big", "fft_are", "fft_aim",
               "fft_naim", "fft_iare", "fft_iaim", "fft_niaim", "mlp1a",
               "mlp1b", "mlp2", "compR", "compI", "cs16"}
    for name, shape in [
        ("xb", (L, DIM)), ("w_in_t", (2, 128, 2 * D_INNER)),
        ("w_dtraw_t", (NDT, 128, DT_RANK)), ("w_b_t", (NDT, 128, SLOC)),
        ("w_c_t", (NDT, 128, SLOC)), ("w_dt_t", (DT_RANK, D_INNER)),
        ("w_out_t", (NDT, 128, DIM)),
        ("conv_diag", (NDT, D_CONV, 128, 128)),
        ("conv_b", (NDT, 128, 1)), ("dt_b", (NDT, 128, 1)), ("d_q", (NDT, 128, 1)),
        ("norm1_w_bc", (128, DIM)), ("norm1_b_bc", (128, DIM)),
        ("norm2_w_bc", (128, DIM)), ("norm2_b_bc", (128, DIM)),
        ("ident128", (128, 128)), ("neg_s", (128, SLOC)),
        ("sel16", (SLOC, SLOC * 128)),
        ("g_big", (64, 128)),
        ("fft_are", (16, 128, 128)), ("fft_aim", (16, 128, 128)),
        ("fft_naim", (16, 128, 128)), ("fft_iare", (16, 128, 128)),
        ("fft_iaim", (16, 128, 128)), ("fft_niaim", (16, 128, 128)),
        ("mlp1a", (NB, 128, 128)), ("mlp1b", (NB, 128, 128)),
        ("mlp2", (NB, 128, 128)),
        ("b1s", (NB, 128, 1)), ("b2p", (NB, 128, 1)), ("b2n", (NB, 128, 1)),
        ("compR", (NB, 128, 64)), ("compI", (NB, 128, 64)),
        ("cs16", (32, 16)),
    ]:
        dt_ = FP16 if name in FP16_IN else (BF16 if name in BF16_IN else FP)
        I[name] = nc.dram_tensor(name, list(shape), dt_, kind="ExternalInput")

    oblk = nc.dram_tensor("oblk", [L, BS], FP, kind="ExternalOutput")
    osumA = nc.dram_tensor("osumA", [L, DIM], BF16, kind="ExternalOutput")
    osumB = nc.dram_tensor("osumB", [L, DIM], BF16, kind="ExternalOutput")

    with tile.TileContext(nc, pool_alloc_mode="queue") as tc:
        _build_body(nc, tc, I, oblk, osumA, osumB)

    _split_multiwait_drains(nc)
    return nc


def _build_body(nc, tc, I, oblk, osumA, osumB):
    from contextlib import ExitStack
    ctx = ExitStack()
    with ctx:
        persist = ctx.enter_context(tc.tile_pool(name="persist", bufs=1))
        dram = ctx.enter_context(tc.tile_pool(name="dram", bufs=1, space="DRAM"))
        scr = ctx.enter_context(tc.tile_pool(name="scr", bufs=2))

        # ---------- persistent tiles ----------
        ln1w = persist.tile([128, DIM], FP16, name="ln1w", tag="ln1w")
        nc.sync.dma_start(ln1w[:], I["norm1_w_bc"][:])
        ln1b = persist.tile([128, DIM], FP16, name="ln1b", tag="ln1b")
        nc.sync.dma_start(ln1b[:], I["norm1_b_bc"][:])
        neg_s = persist.tile([128, SLOC], FP, name="neg_s", tag="neg_s")
        nc.sync.dma_start(neg_s[:], I["neg_s"][:])
        sel16 = persist.tile([SLOC, SLOC * 128], FP16, name="sel16", tag="sel16")
        nc.sync.dma_start(sel16[:], I["sel16"][:])
        bt_sb = persist.tile([SLOC, L], FP16, name="bt", tag="bt")
        ct_sb = persist.tile([SLOC, L], FP16, name="ct", tag="ct")
        eps_t = persist.tile([128, 1], FP, name="eps_t", tag="eps_t")
        nc.vector.memset(eps_t[:], EPS)
        ones_c = persist.tile([128, 1], FP, name="ones_c", tag="ones_c")
        nc.vector.memset(ones_c[:], 1.0)



        xp_big = [persist.tile([128, 8 * DIM], FP, name=f"xpb{c}",
                               tag=f"xpb{c}") for c in range(2)]
        xp_tiles = [xp_big[ti // 8][:, (ti % 8) * DIM:(ti % 8 + 1) * DIM]
                    for ti in range(NTT)]

        o_bnA = dram.tile([L, DIM], BF16, name="o_bnA")
        o_bnB = [dram.tile([L // 2, DIM], BF16, name=f"o_bnB{c}")
                 for c in range(2)]

        # per-d-tile working set for the scan phase
        pmain_cm = tc.tile_pool(name="pmain", bufs=1)
        pmain = pmain_cm.__enter__()
        pxc_cm = tc.tile_pool(name="pxc", bufs=1)
        pxc = pxc_cm.__enter__()
        xc_t = [pxc.tile([128, L], FP16, name=f"xc{i}", tag=f"xc{i}")
                for i in range(NDT)]
        sz_t = [pmain.tile([128, L], FP16, name=f"sz{i}", tag=f"sz{i}")
                for i in range(NDT)]
        dt_t = [pmain.tile([128, L], FP16, name=f"dt{i}", tag=f"dt{i}")
                for i in range(NDT)]
        u_t = [pmain.tile([128, L], FP16, name=f"u{i}", tag=f"u{i}")
               for i in range(NDT)]
        y_t = [pmain.tile([128, L], FP16, name=f"y{i}", tag=f"y{i}")
               for i in range(NDT)]

        # ======== front-end: LN1, transpose, in_proj, conv ========
        fepsA_cm = tc.tile_pool(name="fepsA", bufs=2, space="PSUM")
        fepsA = fepsA_cm.__enter__()
        pt_cm = tc.tile_pool(name="pt", bufs=1)
        ptp = pt_cm.__enter__()
        xnT = [ptp.tile([128, L], FP16, name=f"xnT{i}", tag=f"xnT{i}")
               for i in range(2)]
        with tc.tile_pool(name="pln", bufs=1) as pln:
            x_tiles = [pln.tile([128, DIM], FP, name=f"x{i}", tag=f"x{i}")
                       for i in range(NTT)]
            for ti in range(NTT):
                nc.sync.dma_start(x_tiles[ti][:],
                                  I["xb"][ti * 128:(ti + 1) * 128, :])
            xn_tiles = [pln.tile([128, DIM], FP16, name=f"xn{i}", tag=f"xn{i}")
                        for i in range(NTT)]
            _ln_tiles(nc, x_tiles, ln1w, ln1b, xn_tiles, scr, eps_t, FP16)
            ident = pln.tile([128, 128], FP16, name="ident", tag="ident")
            nc.sync.dma_start(ident[:], I["ident128"][:])
            for ti in range(NTT):
                for cb in range(2):
                    pt = fepsA.tile([128, 128], FP16, name="tpose", tag="tpose")
                    nc.tensor.transpose(
                        pt[:], xn_tiles[ti][:, cb * 128:(cb + 1) * 128], ident[:])
                    nc.scalar.copy(xnT[cb][:, ti * 128:(ti + 1) * 128], pt[:])

        with tc.tile_pool(name="pip", bufs=1) as pip:
            w_in = [pip.tile([128, 2 * D_INNER], FP16, name=f"win{k}",
                             tag=f"win{k}") for k in range(2)]
            for k in range(2):
                nc.sync.dma_start(w_in[k][:], I["w_in_t"][k])
            xi_pad = [pip.tile([128, L + D_CONV - 1], FP16, name=f"xip{i}",
                               tag=f"xip{i}") for i in range(NDT)]
            for i in range(NDT):
                nc.vector.memset(xi_pad[i][:, 0:D_CONV - 1], 0.0)
            for m in range(8):
                for n in range(4):
                    ps = fepsA.tile([128, 512], FP, name="inproj", tag="inproj")
                    for k in range(2):
                        nc.tensor.matmul(ps[:], w_in[k][:, m * 128:(m + 1) * 128],
                                         xnT[k][:, n * 512:(n + 1) * 512],
                                         start=(k == 0), stop=(k == 1))
                    if m < NDT:
                        nc.vector.tensor_copy(
                            xi_pad[m][:, D_CONV - 1 + n * 512:
                                      D_CONV - 1 + (n + 1) * 512], ps[:])
                    else:
                        nc.scalar.activation(
                            sz_t[m - NDT][:, n * 512:(n + 1) * 512], ps[:],
                            AF.Silu)

            cwd = [[pip.tile([128, 128], FP16, name=f"cwd{i}_{k}",
                             tag=f"cwd{i}_{k}") for k in range(D_CONV)]
                   for i in range(NDT)]
            cb_ = [pip.tile([128, 1], FP, name=f"cb{i}", tag=f"cb{i}")
                   for i in range(NDT)]
            for i in range(NDT):
                for k in range(D_CONV):
                    nc.sync.dma_start(cwd[i][k][:], I["conv_diag"][i][k])
                nc.sync.dma_start(cb_[i][:], I["conv_b"][i])
            # depthwise conv on PE: 4 shifted diagonal matmuls per chunk
            with tc.tile_pool(name="cvps", bufs=2, space="PSUM") as cvps:
                for i in range(NDT):
                    for n in range(4):
                        ps = cvps.tile([128, 512], FP, name="cvp", tag="cvp")
                        for k in range(D_CONV):
                            nc.tensor.matmul(
                                ps[:], cwd[i][k][:],
                                xi_pad[i][:, k + n * 512:k + n * 512 + 512],
                                start=(k == 0), stop=(k == D_CONV - 1))
                        nc.scalar.activation(
                            xc_t[i][:, n * 512:(n + 1) * 512], ps[:],
                            AF.Silu, bias=cb_[i][:])
        pt_cm.__exit__(None, None, None)   # free xnT
        fepsA_cm.__exit__(None, None, None)

        # ======== x_proj, dt_proj, u, y-init ========
        fepsB_cm = tc.tile_pool(name="fepsB", bufs=2, space="PSUM")
        fepsB = fepsB_cm.__enter__()
        with tc.tile_pool(name="pfx", bufs=1) as pfx:
            wdtr = [pfx.tile([128, DT_RANK], FP16, name=f"wdtr{k}",
                             tag=f"wdtr{k}") for k in range(NDT)]
            wbt = [pfx.tile([128, SLOC], FP16, name=f"wbt{k}", tag=f"wbt{k}")
                   for k in range(NDT)]
            wct = [pfx.tile([128, SLOC], FP16, name=f"wct{k}", tag=f"wct{k}")
                   for k in range(NDT)]
            for k in range(NDT):
                nc.sync.dma_start(wdtr[k][:], I["w_dtraw_t"][k])
                nc.sync.dma_start(wbt[k][:], I["w_b_t"][k])
                nc.sync.dma_start(wct[k][:], I["w_c_t"][k])
            dtraw = pfx.tile([DT_RANK, L], FP16, name="dtraw", tag="dtraw")
            for n in range(4):
                psd = fepsB.tile([DT_RANK, 512], FP, name="xprojD", tag="xprojD")
                psb = fepsB.tile([DT_RANK, 512], FP, name="xprojB", tag="xprojB")
                psc = fepsB.tile([DT_RANK, 512], FP, name="xprojC", tag="xprojC")
                for k in range(NDT):
                    xck = xc_t[k][:, n * 512:(n + 1) * 512]
                    nc.tensor.matmul(psd[:], wdtr[k][:], xck,
                                     start=(k == 0), stop=(k == NDT - 1))
                    nc.tensor.matmul(psb[:], wbt[k][:], xck,
                                     start=(k == 0), stop=(k == NDT - 1))
                    nc.tensor.matmul(psc[:], wct[k][:], xck,
                                     start=(k == 0), stop=(k == NDT - 1))
                nc.scalar.copy(dtraw[:, n * 512:(n + 1) * 512], psd[:])
                nc.scalar.copy(bt_sb[:, n * 512:(n + 1) * 512], psb[:])
                nc.scalar.copy(ct_sb[:, n * 512:(n + 1) * 512], psc[:])

            wdt = pfx.tile([DT_RANK, D_INNER], FP16, name="wdt", tag="wdt")
            nc.sync.dma_start(wdt[:], I["w_dt_t"][:])
            dtb = [pfx.tile([128, 1], FP, name=f"dtb{i}", tag=f"dtb{i}")
                   for i in range(NDT)]
            for i in range(NDT):
                nc.sync.dma_start(dtb[i][:], I["dt_b"][i])
            for i in range(NDT):
                for n in range(4):
                    ps = fepsB.tile([128, 512], FP, name="dtproj", tag="dtproj")
                    nc.tensor.matmul(ps[:], wdt[:, i * 128:(i + 1) * 128],
                                     dtraw[:, n * 512:(n + 1) * 512],
                                     start=True, stop=True)
                    # softplus: dt = ln(1 + exp(raw + b))
                    t1 = pfx.tile([128, 512], FP, name="sp_t1", tag="sp_t1",
                                  bufs=2)
                    nc.scalar.activation(t1[:], ps[:], AF.Exp, bias=dtb[i][:])
                    nc.scalar.activation(dt_t[i][:, n * 512:(n + 1) * 512],
                                         t1[:], AF.Ln, bias=ones_c[:])

            dq = [pfx.tile([128, 1], FP, name=f"dq{i}", tag=f"dq{i}")
                  for i in range(NDT)]
            for i in range(NDT):
                nc.sync.dma_start(dq[i][:], I["d_q"][i])
            for i in range(NDT):
                nc.vector.tensor_mul(u_t[i][:], dt_t[i][:], xc_t[i][:])
                nc.vector.tensor_scalar(y_t[i][:], xc_t[i][:], dq[i][:],
                                        None, ALU.mult)
        fepsB_cm.__exit__(None, None, None)
        pxc_cm.__exit__(None, None, None)   # free xc
        py2_cm = tc.tile_pool(name="py2", bufs=1)
        py2 = py2_cm.__enter__()
        y2_t = [py2.tile([128, L], FP16, name=f"y2_{i}", tag=f"y2_{i}")
                for i in range(NDT)]

        # FFT-phase constants: issued here so the DMAs hide under the scan
        ln2w = persist.tile([128, DIM], BF16, name="ln2w", tag="ln2w")
        nc.sync.dma_start(ln2w[:], I["norm2_w_bc"][:])
        ln2b = persist.tile([128, DIM], BF16, name="ln2b", tag="ln2b")
        nc.sync.dma_start(ln2b[:], I["norm2_b_bc"][:])

        # resident constants
        gbig = persist.tile([64, 128], BF16, name="gbig", tag="gbig")
        nc.sync.dma_start(gbig[:], I["g_big"][:])
        fare = persist.tile([128, 16 * 128], BF16, name="fare", tag="fare")
        faim = persist.tile([128, 16 * 128], BF16, name="faim", tag="faim")
        fnaim = persist.tile([128, 16 * 128], BF16, name="fnaim", tag="fnaim")
        fiare = persist.tile([128, 16 * 128], BF16, name="fiare", tag="fiare")
        fiaim = persist.tile([128, 16 * 128], BF16, name="fiaim", tag="fiaim")
        fniaim = persist.tile([128, 16 * 128], BF16, name="fniaim", tag="fniaim")
        for r in range(16):
            sl = slice(r * 128, (r + 1) * 128)
            nc.sync.dma_start(fare[:, sl], I["fft_are"][r])
            nc.sync.dma_start(faim[:, sl], I["fft_aim"][r])
            nc.sync.dma_start(fnaim[:, sl], I["fft_naim"][r])
            nc.sync.dma_start(fiare[:, sl], I["fft_iare"][r])
            nc.sync.dma_start(fiaim[:, sl], I["fft_iaim"][r])
            nc.sync.dma_start(fniaim[:, sl], I["fft_niaim"][r])
        m1a = [persist.tile([128, 128], BF16, name=f"m1a{k}", tag=f"m1a{k}")
               for k in range(NB)]
        m1b = [persist.tile([128, 128], BF16, name=f"m1b{k}", tag=f"m1b{k}")
               for k in range(NB)]
        m2 = [persist.tile([128, 128], BF16, name=f"m2_{k}", tag=f"m2_{k}")
              for k in range(NB)]
        b1s = [persist.tile([128, 1], FP, name=f"b1s{k}", tag=f"b1s{k}")
               for k in range(NB)]
        b2p = [persist.tile([128, 1], FP, name=f"b2p{k}", tag=f"b2p{k}")
               for k in range(NB)]
        b2n = [persist.tile([128, 1], FP, name=f"b2n{k}", tag=f"b2n{k}")
               for k in range(NB)]
        cmpR = [persist.tile([128, 64], BF16, name=f"cmR{k}", tag=f"cmR{k}")
                for k in range(NB)]
        cmpI = [persist.tile([128, 64], BF16, name=f"cmI{k}", tag=f"cmI{k}")
                for k in range(NB)]
        for k in range(NB):
            nc.sync.dma_start(m1a[k][:], I["mlp1a"][k])
            nc.sync.dma_start(m1b[k][:], I["mlp1b"][k])
            nc.sync.dma_start(m2[k][:], I["mlp2"][k])
            nc.sync.dma_start(b1s[k][:], I["b1s"][k])
            nc.sync.dma_start(b2p[k][:], I["b2p"][k])
            nc.sync.dma_start(b2n[k][:], I["b2n"][k])
            nc.sync.dma_start(cmpR[k][:], I["compR"][k])
            nc.sync.dma_start(cmpI[k][:], I["compI"][k])
        cs16 = persist.tile([32, 16], BF16, name="cs16", tag="cs16")
        nc.sync.dma_start(cs16[:], I["cs16"][:])

        # ================= scan phase (si-split AllReduce) =================
        wout = [persist.tile([128, DIM], FP16, name=f"wout{k}", tag=f"wout{k}")
                for k in range(NDT)]
        for k in range(NDT):
            nc.sync.dma_start(wout[k][:], I["w_out_t"][k])
        o_sA = dram.tile([L, DIM], BF16, name="o_sA")
        o_sB = [dram.tile([L // 2, DIM], BF16, name=f"o_sB{c}")
                for c in range(2)]

        with tc.tile_pool(name="scn", bufs=2) as sp, \
             tc.tile_pool(name="scn1", bufs=1) as sp1, \
             tc.tile_pool(name="scnps", bufs=2, space="PSUM") as sps, \
             tc.tile_pool(name="arps", bufs=2, space="PSUM") as arps:
            for si in range(SLOC):
                bbc16 = sp.tile([128, L], FP16, name="bbc16", tag="bbc16")
                cbc16 = sp.tile([128, L], FP16, name="cbc16", tag="cbc16")
                selw = sel16[:, si * 128:(si + 1) * 128]
                for n in range(4):
                    csl = slice(n * 512, (n + 1) * 512)
                    bps = sps.tile([128, 512], FP, name="bps", tag="bps")
                    nc.tensor.matmul(bps[:], selw, bt_sb[:, csl],
                                     start=True, stop=True)
                    nc.scalar.copy(bbc16[:, csl], bps[:])
                    cps = sps.tile([128, 512], FP, name="cps", tag="cps")
                    nc.tensor.matmul(cps[:], selw, ct_sb[:, csl],
                                     start=True, stop=True)
                    nc.scalar.copy(cbc16[:, csl], cps[:])
                # group A: si 0..7 (y_t, includes D-term init); group B:
                # si 8..15 (y2_t).
                yt = y_t if si < 8 else y2_t
                for i in range(NDT):
                    dA = sp.tile([128, L], FP16, name="dA", tag="dA")
                    nc.scalar.activation(dA[:], dt_t[i][:], AF.Exp,
                                         scale=neg_s[:, si:si + 1])
                    dBu = sp1.tile([128, L], FP16, name="dBu", tag="dBu")
                    nc.vector.tensor_mul(dBu[:], u_t[i][:], bbc16[:])
                    h = sp1.tile([128, L], FP16, name="h", tag="h")
                    nc.vector.tensor_tensor_scan(h[:], dA[:], dBu[:], 0.0,
                                                 ALU.mult, ALU.add)
                    if si == 8:
                        nc.vector.tensor_mul(yt[i][:], h[:], cbc16[:])
                    else:
                        w = sp1.tile([128, L], FP16, name="w", tag="w")
                        nc.vector.tensor_mul(w[:], h[:], cbc16[:])
                        nc.vector.tensor_add(yt[i][:], yt[i][:], w[:])
                if si == 7:
                    # group-A gate + out_proj + AllReduce; the collective
                    # runs during the si=8..15 scanning.
                    for i in range(NDT):
                        nc.vector.tensor_mul(y_t[i][:], y_t[i][:],
                                             sz_t[i][:])
                    for ti in range(NTT):
                        ps = arps.tile([128, DIM], FP, name="oprojA",
                                       tag="oprojA")
                        for k in range(NDT):
                            nc.tensor.matmul(
                                ps[:], y_t[k][:, ti * 128:(ti + 1) * 128],
                                wout[k][:], start=(k == 0),
                                stop=(k == NDT - 1))
                        ot = scr.tile([128, DIM], BF16, name="oevA",
                                      tag="oevA")
                        nc.scalar.copy(ot[:], ps[:])
                        nc.sync.dma_start(
                            o_bnA[ti * 128:(ti + 1) * 128, :], ot[:])
                    nc.gpsimd.collective_compute(
                        "AllReduce", ALU.add,
                        replica_groups=[[0, 1, 2, 3], [4, 5, 6, 7]],
                        ins=[o_bnA.opt()], outs=[o_sA.opt()])
                    nc.sync.dma_start(osumA[:], o_sA[:])

        py2_cm.__exit__(None, None, None)   # free y2 (group B projected)

        # ==== group-B gate + out_proj + per-half AllReduce; xp assembly ====
        with tc.tile_pool(name="postps", bufs=2, space="PSUM") as pops, \
             tc.tile_pool(name="pxa", bufs=1) as pxa:
            for i in range(NDT):
                nc.vector.tensor_mul(y2_t[i][:], y2_t[i][:], sz_t[i][:])
            for c in range(2):
                hsl = slice(c * 1024, (c + 1) * 1024)
                for tj in range(8):
                    ti = c * 8 + tj
                    ps = pops.tile([128, DIM], FP, name="oproj", tag="oproj")
                    for k in range(NDT):
                        nc.tensor.matmul(
                            ps[:], y2_t[k][:, ti * 128:(ti + 1) * 128],
                            wout[k][:], start=(k == 0), stop=(k == NDT - 1))
                    ot = scr.tile([128, DIM], BF16, name="oev", tag="oev")
                    nc.scalar.copy(ot[:], ps[:])
                    nc.sync.dma_start(o_bnB[c][tj * 128:(tj + 1) * 128, :],
                                      ot[:])
                nc.gpsimd.collective_compute(
                    "AllReduce", ALU.add,
                    replica_groups=[[0, 1, 2, 3], [4, 5, 6, 7]],
                    ins=[o_bnB[c].opt()], outs=[o_sB[c].opt()])
                nc.sync.dma_start(osumB[hsl, :], o_sB[c][:])
                # xp half = x + osumA + osumB (bulk rearranged-AP loads)
                ld = pxa.tile([128, 8 * DIM], BF16, name="ld", tag="ld")
                nc.sync.dma_start(
                    ld[:].rearrange("p (t d) -> p t d", t=8),
                    o_sA[hsl, :].rearrange("(t p) d -> p t d", p=128))
                t1 = pxa.tile([128, 8 * DIM], FP, name="t1", tag="t1")
                nc.scalar.copy(t1[:], ld[:])
                ld2 = pxa.tile([128, 8 * DIM], BF16, name="ld2", tag="ld2")
                nc.sync.dma_start(
                    ld2[:].rearrange("p (t d) -> p t d", t=8),
                    o_sB[c][:].rearrange("(t p) d -> p t d", p=128))
                t2 = pxa.tile([128, 8 * DIM], FP, name="t2", tag="t2")
                nc.scalar.copy(t2[:], ld2[:])
                nc.vector.tensor_add(t1[:], t1[:], t2[:])
                nc.sync.dma_start(
                    t2[:].rearrange("p (t d) -> p t d", t=8),
                    I["xb"][hsl, :].rearrange("(t p) d -> p t d", p=128))
                nc.vector.tensor_add(xp_big[c][:], t2[:], t1[:])
        pmain_cm.__exit__(None, None, None)   # free sz/dt/u/y

        # ================= LN2 + EinFFT (bf16, SBUF-resident) =================
        with tc.tile_pool(name="fft", bufs=1) as ff, \
             tc.tile_pool(name="fftsc", bufs=2) as fsc:

            pack = ff.tile([64, 128 * BS], BF16, name="pack", tag="pack")
            gfull = ff.tile([128, 128 * BS], BF16, name="gfull", tag="gfull")
            ftre = [ff.tile([128, L], BF16, name=f"ftre{u}", tag=f"ftre{u}")
                    for u in range(2)]
            ftim = [ff.tile([128, L], BF16, name=f"ftim{u}", tag=f"ftim{u}")
                    for u in range(2)]
            z2st = [ff.tile([128, L], BF16, name=f"z2st{k}", tag=f"z2st{k}")
                    for k in range(NB)]
            wpack = ff.tile([32, 128 * BS], BF16, name="wpack", tag="wpack")

            fpsG_cm = tc.tile_pool(name="fpsG", bufs=2, space="PSUM")
            fpsG = fpsG_cm.__enter__()
            # --- LN2 + pack (token-transpose via SBUF->SBUF DMA)
            with tc.tile_pool(name="fftln", bufs=1) as fln:
                xn2 = [fln.tile([128, DIM], BF16, name=f"xn2_{i}",
                                tag=f"xn2_{i}") for i in range(NTT)]
                _ln_tiles(nc, xp_tiles, ln2w, ln2b, xn2, scr, eps_t, BF16)
                for a in range(16):
                    for k in range(NB):
                        row = 4 * a + k
                        eng = (nc.sync, nc.scalar)[row % 2]
                        eng.dma_start(pack[row:row + 1, :],
                                      xn2[a][:, 64 * k:64 * (k + 1)])

                # --- G: (a,k) -> (kf,r) mixing; out rows = [re(64); im(64)]
                gfull3 = gfull[:].rearrange("m (d s) -> m d s", d=64)
                for n in range(16):
                    sl = slice(n * 512, (n + 1) * 512)
                    psg = fpsG.tile([128, 512], FP, name="gps", tag="gps")
                    nc.tensor.matmul(psg[:], gbig[:], pack[:, sl],
                                     start=True, stop=True)
                    nc.vector.tensor_copy(
                        gfull3[:, :, 8 * n:8 * (n + 1)],
                        psg[:].rearrange("m (s d) -> m d s", s=8))

            # --- E4 input: XBAR DMA transpose (one instruction per half)
            # packT_*[s, d, (16kf+r)] = G[(kf,r), (s,d)]
            packT_re = ff.tile([128, 4096], BF16, name="packT_re",
                               tag="packT_re")
            packT_im = ff.tile([128, 4096], BF16, name="packT_im",
                               tag="packT_im")
            nc.sync.dma_start_transpose(
                packT_re[:].rearrange("p (e c) -> p e c", e=64),
                gfull[0:64, :])
            nc.sync.dma_start_transpose(
                packT_im[:].rearrange("p (e c) -> p e c", e=64),
                gfull[64:128, :])
            vre = packT_re[:].rearrange("p (d c) -> p c d", d=64)
            vim = packT_im[:].rearrange("p (d c) -> p c d", d=64)
            gtr_r = [ff.tile([128, 256], BF16, name=f"gtr{r}", tag=f"gtr{r}")
                     for r in range(16)]
            gti_r = [ff.tile([128, 256], BF16, name=f"gti{r}", tag=f"gti{r}")
                     for r in range(16)]
            for r in range(16):
                nc.vector.tensor_copy(gtr_r[r][:], vre[:, r::16, :])
                nc.vector.tensor_copy(gti_r[r][:], vim[:, r::16, :])

            # --- E4 / E5 / E6 / E7 interleaved by r-group so the PE,
            # Scalar, and Vector queues pipeline across stages. Every
            # accumulation group gets its own PSUM tile (banks must not be
            # shared between concurrent accumulation groups). The G-stage
            # pool is closed here so E4 and the MLP can double-buffer.
            fpsG_cm.__exit__(None, None, None)
            fps4_cm = tc.tile_pool(name="fps4", bufs=2, space="PSUM")
            fps4 = fps4_cm.__enter__()
            fps5_cm = tc.tile_pool(name="fps5", bufs=2, space="PSUM")
            fps5 = fps5_cm.__enter__()
            fps6_cm = tc.tile_pool(name="fps6", bufs=1, space="PSUM")
            fps6 = fps6_cm.__enter__()
            for n in range(4):
                for r in range(4 * n, 4 * n + 4):
                    rsl = slice(r * 128, (r + 1) * 128)
                    gtr, gti = gtr_r[r], gti_r[r]
                    for u in range(2):
                        lsl = slice(u * 128, (u + 1) * 128)
                        pre = fps4.tile([128, 128], FP, name="e4ps",
                                        tag="e4ps")
                        nc.tensor.matmul(pre[:], gtr[:, lsl], fare[:, rsl],
                                         start=True, stop=False)
                        nc.tensor.matmul(pre[:], gti[:, lsl], fnaim[:, rsl],
                                         start=False, stop=True)
                        nc.vector.tensor_copy(ftre[u][:, rsl], pre[:])
                        pim = fps4.tile([128, 128], FP, name="e4ps",
                                        tag="e4ps")
                        nc.tensor.matmul(pim[:], gtr[:, lsl], faim[:, rsl],
                                         start=True, stop=False)
                        nc.tensor.matmul(pim[:], gti[:, lsl], fare[:, rsl],
                                         start=False, stop=True)
                        nc.vector.tensor_copy(ftim[u][:, rsl], pim[:])
                sl = slice(n * 512, (n + 1) * 512)
                for kf in range(NB):
                    u, lo = kf // 2, 64 * (kf % 2)
                    fre = ftre[u][lo:lo + 64, :]
                    fim = ftim[u][lo:lo + 64, :]
                    l1 = fps5.tile([128, 512], FP, name="mlp1", tag="mlp")
                    nc.tensor.matmul(l1[:], m1a[kf][lo:lo + 64, :],
                                     fre[:, sl], start=True, stop=False)
                    nc.tensor.matmul(l1[:], m1b[kf][lo:lo + 64, :],
                                     fim[:, sl], start=False, stop=True)
                    r1i1 = fsc.tile([128, 512], BF16, name="r1i1",
                                    tag="r1i1")
                    nc.scalar.activation(r1i1[:], l1[:], AF.Relu,
                                         bias=b1s[kf][:])
                    l2 = fps5.tile([128, 512], FP, name="mlp2", tag="mlp")
                    nc.tensor.matmul(l2[:], m2[kf][:], r1i1[:],
                                     start=True, stop=True)
                    shp = fsc.tile([128, 512], BF16, name="shp", tag="shp")
                    nc.scalar.activation(shp[:], l2[:], AF.Relu,
                                         bias=b2p[kf][:])
                    shn = fsc.tile([128, 512], BF16, name="shn", tag="shn")
                    nc.scalar.activation(shn[:], l2[:], AF.Relu,
                                         bias=b2n[kf][:], scale=-1.0)
                    nc.vector.tensor_sub(z2st[kf][:, sl], shp[:], shn[:])
                for r in range(4 * n, 4 * n + 4):
                    rsl = slice(r * 128, (r + 1) * 128)
                    z3r = fps6.tile([128, BS], FP, name="e6ps", tag="e6ps")
                    z3i = fps6.tile([128, BS], FP, name="e6ps2",
                                    tag="e6ps2")
                    for kf in range(NB):
                        nc.tensor.matmul(z3r[:], z2st[kf][:, rsl],
                                         cmpR[kf][:], start=(kf == 0),
                                         stop=(kf == NB - 1))
                        nc.tensor.matmul(z3i[:], z2st[kf][:, rsl],
                                         cmpI[kf][:], start=(kf == 0),
                                         stop=(kf == NB - 1))
                    z3r_sb = fsc.tile([128, BS], BF16, name="z3sb",
                                      tag="z3sb")
                    nc.scalar.copy(z3r_sb[:], z3r[:])
                    z3i_sb = fsc.tile([128, BS], BF16, name="z3sb2",
                                      tag="z3sb2")
                    nc.scalar.copy(z3i_sb[:], z3i[:])
                    pw = fps6.tile([128, BS], FP, name="e7ps", tag="e7ps")
                    nc.tensor.matmul(pw[:], fiare[:, rsl], z3r_sb[:],
                                     start=True, stop=False)
                    nc.tensor.matmul(pw[:], fniaim[:, rsl], z3i_sb[:],
                                     start=False, stop=True)
                    wev = fsc.tile([128, BS], BF16, name="wev", tag="wev")
                    nc.scalar.copy(wev[:], pw[:])
                    nc.sync.dma_start(wpack[r:r + 1, :], wev[:])
                    pw2 = fps6.tile([128, BS], FP, name="e7ps2",
                                    tag="e7ps")
                    nc.tensor.matmul(pw2[:], fiaim[:, rsl], z3r_sb[:],
                                     start=True, stop=False)
                    nc.tensor.matmul(pw2[:], fiare[:, rsl], z3i_sb[:],
                                     start=False, stop=True)
                    wev2 = fsc.tile([128, BS], BF16, name="wev", tag="wev")
                    nc.scalar.copy(wev2[:], pw2[:])
                    nc.sync.dma_start(wpack[16 + r:16 + r + 1, :], wev2[:])
            fps6_cm.__exit__(None, None, None)
            fps5_cm.__exit__(None, None, None)
            fps4_cm.__exit__(None, None, None)
            fps8_cm = tc.tile_pool(name="fps8", bufs=2, space="PSUM")
            fps8 = fps8_cm.__enter__()
            # --- E8: final recombination over r; stream chunks to oblk
            oblk_v = oblk[:].rearrange("(a p) d -> a p d", a=16)
            for n in range(16):
                sl = slice(n * 512, (n + 1) * 512)
                ps = fps8.tile([16, 512], FP, name="eops", tag="eops")
                nc.tensor.matmul(ps[:], cs16[:], wpack[:, sl],
                                 start=True, stop=True)
                ev = fsc.tile([16, 512], FP, name="eoev", tag="eoev")
                nc.scalar.copy(ev[:], ps[:])
                nc.sync.dma_start(oblk_v[:, 8 * n:8 * (n + 1), :], ev[:])
            fps8_cm.__exit__(None, None, None)


# --------------------------------------------------------------------------
# host entry point
# --------------------------------------------------------------------------

_CACHE = {}


def _get_program(debug=False):
    key = ("prog", debug)
    if key not in _CACHE:
        _CACHE[key] = build_program(num_cores=8, debug=debug)
    return _CACHE[key]


def run(inputs, debug=False, trace=False):
    W = {k: np.asarray(v) for k, v in inputs.items() if k not in ("H", "W")}
    shared = _shared_consts(W)
    in_maps = []
    for core in range(8):
        m = dict(shared)
        m.update(_core_consts(W, core))
        in_maps.append(m)
    nc = _get_program(debug=debug)
    res = run_bass_kernel_spmd(nc, in_maps, core_ids=list(range(8)), trace=trace)
    x = np.asarray(W["x"], np.float32)
    out = np.empty((B_SZ, L, DIM), np.float32)
    for core in range(8):
        b, j = core // 4, core % 4
        sl = slice(BS * j, BS * (j + 1))
        r = res.results[core]
        osum = (r["osumA"].astype(np.float32) + r["osumB"].astype(np.float32))
        out[b, :, sl] = x[b][:, sl] + osum[:, sl] + r["oblk"]
    return out, res


def kernel(**inputs):
    out, _ = run(inputs)
    return out
